# revision 52
# baseline (speedup 1.0000x reference)
"""AttrPredLoss_40 focal-BCE loss kernel for Trainium2 (8 NeuronCores, data parallel).

Math (per sample row, 18 selected attrs j):
    p   = pred[:, SEL]                      # in (0,1)
    t   = label in {0,1}
    d   = t - p
    # BCE log arg: t==1 -> p, t==0 -> 1-p  ==  1-|d|
    l   = max(ln(1-|d|), -100)              # sum_j l = -18*bce_mean
    # focal weight: (t?0.8:0.2) * (t?1-p:p)^2 == (t+1/3)*0.6*d^2
    f   = (t + 1/3) * 0.6 * d^2
    loss = sum_rows (sum_j f) * (-1/18) * (sum_j l)

Champion pipeline (BEST_KW): one HWDGE ring streams pred+label at ~345 GB/s
(triple-buffered 128-row tiles, tapered tail); per tile
    DVE: d = t - p (6 merged-run subs), a2 = (t-0.5)*d = |d|/2 (one STT —
         the label IS d's sign), f = (t+1/3)*s6 (STT)
    ACT: s6 = Square(sqrt(.6)*d), l = Ln(-2*a2 + 1 + 2^-23)   [bf16 out]
    PE:  G += f_chunk^T @ l_chunk, diagonal 18x18 blocks in PSUM,
         4 row-groups (72 cols) per matmul, one start/stop chain.
The eps-biased Ln replaces the -100 clamp (p=0 -> ln ~ -15.9, error ~3e-6
of the total); ln(1+eps-|d|) arrives bit-identically via a2.

total = sum over diagonal 18x18 blocks of G, * (-1/18), summed over cores.
"""

import math
from contextlib import ExitStack

import numpy as np

import concourse.bacc as bacc
import concourse.bass as bass
import concourse.mybir as mybir
import concourse.tile as tile
from concourse.bass_utils import run_bass_kernel_spmd

F32 = mybir.dt.float32
I32 = mybir.dt.int32
BF16 = mybir.dt.bfloat16
F16 = mybir.dt.float16
F8 = mybir.dt.float8e4
ALU = mybir.AluOpType
ACTF = mybir.ActivationFunctionType

# selected attribute indices, as contiguous runs: (label_col, pred_col, len)
RUNS = [
    (0, 4, 1),
    (1, 8, 2),
    (3, 11, 1),
    (4, 13, 6),
    (10, 20, 1),
    (11, 22, 2),
    (13, 26, 1),
    (14, 29, 3),
    (17, 36, 1),
]
# same 18 columns as 6 ops: (label_col0, pred_col0, nrun, lstep, pstep, len)
# pairs of runs whose spacing is arithmetic on BOTH the label and pred side
MERGED_RUNS = [
    (0, 4, 1, 0, 0, 1),     # {4}
    (1, 8, 2, 10, 14, 2),   # {8,9} + {22,23}
    (3, 11, 2, 10, 15, 1),  # {11} + {26}
    (4, 13, 1, 0, 0, 6),    # {13..18}
    (10, 20, 2, 7, 16, 1),  # {20} + {36}
    (14, 29, 1, 0, 0, 3),   # {29,30,31}
]
NSEL = 18
NCOL = 40

N_CORES = 8
B = 1_048_576
B_SHARD = B // N_CORES  # 131072
P = 128  # SBUF partitions
U = 4  # row-groups per matmul chunk (must divide r; 18*4 = 72 partitions)
GDIM = NSEL * U  # 72


def build_nc(
    b_shard: int = B_SHARD,
    r: int = 128,
    label_cast: str = "dma",
    loop_n: int = 1,
    io_bufs: int = 2,
    variant: str = "full",
    tile_sizes: list | None = None,
    label_bufs: int | None = None,
    clamp_engine: str = "dve",
    label_engine: str = "sync",
    loop_staggered: bool = False,
    sw_pipe: bool = False,
    relu_clamp: bool = False,
    eps_ln: bool = False,
    act_order: str = "sq_first",
    mid_bufs: int = 2,
    lean: bool = False,
    f_engine: str = "dve",
    gcopy_engine: str = "scalar",
    label_prefetch: bool = False,
    s6_bf16: bool = False,
    warm_ln: bool = False,
    tail_label_prefetch: int = 0,
    u_groups: int = U,
    d_psum: bool = False,
    pe_fp8: bool = False,
    split_g: int = 0,
):
    """Build the per-core Bass module. b_shard rows per core; r rows per
    partition per tile (or explicit tile_sizes list summing to nrows).
    loop_n>1 wraps the body in a device-side For loop (same data re-read
    each iteration) for wall-clock benchmarking."""
    assert b_shard % P == 0
    nrows = b_shard // P
    if tile_sizes is None:
        assert nrows % r == 0
        tile_sizes = [r] * (nrows // r)
    assert sum(tile_sizes) == nrows, (tile_sizes, nrows)
    assert all(t % u_groups == 0 or t >= u_groups for t in tile_sizes)

    nc = bacc.Bacc("TRN2", target_bir_lowering=False, debug=False)

    pred = nc.dram_tensor("pred", [b_shard, NCOL], F32, kind="ExternalInput")
    label = nc.dram_tensor("label", [b_shard, NSEL], I32, kind="ExternalInput")
    gdim = NSEL * u_groups
    if split_g:
        assert not relu_clamp
        gw = 2 * gdim
    else:
        gw = gdim + 1 if relu_clamp else gdim
    gout = nc.dram_tensor("g_out", [gdim, gw], F32, kind="ExternalOutput")

    # blocked layout: partition p holds rows [p*nrows, (p+1)*nrows)
    pred_r = pred.ap().rearrange("(p n) c -> p n c", p=P)
    label_r = label.ap().rearrange("(p n) c -> p n c", p=P)

    s6scale = math.sqrt(0.6)

    with tile.TileContext(nc) as tc, ExitStack() as ctx:
        io = ctx.enter_context(tc.tile_pool(name="io", bufs=io_bufs))
        if label_bufs is None:
            iol = io
        else:
            iol = ctx.enter_context(tc.tile_pool(name="iol", bufs=label_bufs))
        mid = ctx.enter_context(tc.tile_pool(name="mid", bufs=mid_bufs))
        singles = ctx.enter_context(tc.tile_pool(name="singles", bufs=1))
        psum = ctx.enter_context(tc.tile_pool(name="psum", bufs=1, space="PSUM"))
        psum_d = (
            ctx.enter_context(tc.tile_pool(name="psumd", bufs=1, space="PSUM"))
            if d_psum
            else None
        )

        G = psum.tile([gdim, gdim], F32)
        Gb = None
        if split_g:
            Gb = psum.tile([gdim, gdim], F32, tag="Gb")
        G2 = None
        ones1 = None
        if relu_clamp:
            G2 = psum.tile([gdim, 1], F32)
            ones1 = singles.tile([P, 1], F16)
            nc.vector.memset(ones1[:], 1.0)
            c100 = singles.tile([P, 1], F32)
            nc.vector.memset(c100[:], 100.0)
        ceps = None
        if eps_ln:
            ceps = singles.tile([P, 1], F32)
            nc.vector.memset(ceps[:], 1.0 + 2.0**-23)

        total_chunks = sum(-(-t // u_groups) for t in tile_sizes)
        chunk_idx = 0
        split_chunk = (
            total_chunks - sum(-(-t // u_groups) for t in tile_sizes[-split_g:])
            if split_g
            else None
        )

        if loop_n > 1:
            loop_cm = tc.For_i(0, loop_n, 1, staggered_reset=loop_staggered)
        else:
            loop_cm = None
        if loop_cm is not None:
            ctx.enter_context(loop_cm)

        pt_s = tt_s = None
        if variant == "decoupled":
            # static source tiles: compute runs the full chain but never
            # waits on (or releases) the streaming DMA buffers
            rt_max = max(tile_sizes)
            pt_s = singles.tile([P, rt_max, NCOL], F32, tag="pt_s")
            nc.vector.memset(pt_s[:], 0.5)
            tt_s = singles.tile([P, rt_max, NSEL], I32, tag="tt_s")
            nc.vector.memset(tt_s[:], 1)

        if warm_ln:
            # tiny Ln first: its table set (natural_log) also contains
            # square/abs, so the per-iteration LoadActFuncSet pair for
            # Square-set/Ln-set alternation may collapse to none
            wt = singles.tile([P, 1], F32, tag="warm")
            nc.scalar.activation(wt[:], ceps[:, 0:1], ACTF.Ln)

        tlab = None
        if label_prefetch:
            # all labels in ONE leading DMA: 10 fewer stream boundaries and
            # no per-tile label dependency in the pipeline tail
            tlab = singles.tile([P, nrows, NSEL], I32, tag="tlab")
            lab_eng0 = {"sync": nc.sync, "scalar": nc.scalar, "gpsimd": nc.gpsimd}[
                label_engine
            ]
            lab_eng0.dma_start(out=tlab[:], in_=label_r[:, :, :])

        # prefetch the last K tiles' labels at stream start so the tail
        # chains begin the moment their (last-in-stream) pred DMAs land
        pre_tt: dict[int, object] = {}
        if tail_label_prefetch:
            row0_pre = 0
            offs = []
            for i, rt_ in enumerate(tile_sizes):
                offs.append((i, rt_, row0_pre))
                row0_pre += rt_
            for i, rt_, r0 in offs[-tail_label_prefetch:]:
                ptile = singles.tile([P, rt_, NSEL], I32, tag=f"ttpre{i}")
                nc.sync.dma_start(
                    out=ptile[:], in_=label_r[:, slice(r0, r0 + rt_), :]
                )
                pre_tt[i] = ptile

        Gs = singles.tile([gdim, gw], F32, tag="Gs")

        def g_copy(dst, src):
            if gcopy_engine == "vector":
                nc.vector.tensor_copy(out=dst, in_=src)
            else:
                nc.scalar.copy(dst, src)

        def emit_front(rt, rsl, idx=-1):
            """DMA + ops that depend only on this tile's DMA."""
            pt = io.tile([P, rt, NCOL], F32, tag="pred")
            nc.sync.dma_start(out=pt[:], in_=pred_r[:, rsl, :])

            lab_eng = {"sync": nc.sync, "scalar": nc.scalar, "gpsimd": nc.gpsimd}[
                label_engine
            ]
            if idx in pre_tt:
                tt = pre_tt[idx]
            elif label_prefetch:
                tt = tlab[:, rsl, :]
            elif label_cast == "dma":
                tt = iol.tile([P, rt, NSEL], F32, tag="label")
                nc.gpsimd.dma_start(out=tt[:], in_=label_r[:, rsl, :])
            elif label_cast == "none":
                # no cast: DVE ops read the i32 labels with on-read convert.
                # label_engine picks the issuing queue — a non-sync engine
                # overlaps the label stream with the pred stream on SP.
                tt = iol.tile([P, rt, NSEL], I32, tag="label")
                lab_eng.dma_start(out=tt[:], in_=label_r[:, rsl, :])
            else:
                ti = iol.tile([P, rt, NSEL], I32, tag="label_i")
                nc.sync.dma_start(out=ti[:], in_=label_r[:, rsl, :])
                tt = iol.tile([P, rt, NSEL], F32, tag="label")
                nc.gpsimd.tensor_copy(
                    out=tt.rearrange("p n c -> p (n c)"),
                    in_=ti.rearrange("p n c -> p (n c)"),
                )

            if variant == "dma_only":
                return None

            if variant == "decoupled":
                pt = pt_s[:, 0:rt, :]
                tt = tt_s[:, 0:rt, :]

            # d = t - p on the 18 selected columns (6 merged-run ops)
            d = (psum_d if d_psum else mid).tile([P, rt, NSEL], F32, tag="d")

            def run_view(tile3, col0, nrun, step, ln_):
                base = tile3 if isinstance(tile3, bass.AP) else tile3[:]
                ap = [base.ap[0], base.ap[1]]
                if nrun > 1:
                    ap.append([step, nrun])
                ap.append([1, ln_])
                return bass.AP(tensor=base.tensor, offset=base.offset + col0, ap=ap)

            for lc0, pc0, nrun, lstep, pstep, ln_ in MERGED_RUNS:
                nc.vector.tensor_sub(
                    run_view(d, lc0, nrun, lstep, ln_),
                    run_view(tt, lc0, nrun, lstep, ln_),
                    run_view(pt, pc0, nrun, pstep, ln_),
                )

            if variant == "dpass":
                return None

            dflat = d.rearrange("p n c -> p (n c)")

            if lean:
                # a2 = (t - 0.5)*d = |d|/2 in ONE STT op (the label's value
                # IS d's sign: t=1 -> d>=0, t=0 -> d<=0). Ln's affine
                # prestage absorbs the *2: Ln(-2*a2 + 1 + eps) = Ln(1+eps-|d|)
                # — the ACT Abs pass disappears, numerics unchanged.
                assert eps_ln
                ttf = tt.rearrange("p n c -> p (n c)")
                s6 = mid.tile([P, rt * NSEL], BF16 if s6_bf16 else F32, tag="s6")
                if lean == "dvesq":
                    # s6' = d*d on DVE (ACT down to the single Ln pass);
                    # the 0.6 factor moves to the host-side block sum
                    nc.vector.tensor_tensor(s6[:], dflat, dflat, ALU.mult)
                else:
                    nc.scalar.activation(s6[:], dflat, ACTF.Square, scale=s6scale)
                a2 = mid.tile([P, rt * NSEL], F32, tag="a2")
                a2_eng = nc.gpsimd if lean == "pool" else nc.vector
                a2_eng.scalar_tensor_tensor(
                    a2[:], ttf, -0.5, dflat, ALU.add, ALU.mult
                )
                l = mid.tile([P, rt * NSEL], F8 if pe_fp8 else BF16, tag="l")
                nc.scalar.activation(
                    l[:], a2[:], ACTF.Ln, bias=ceps[:, 0:1], scale=-2.0
                )
                if variant == "acts":
                    return None
                return (rt, tt, s6, l)

            s6 = mid.tile([P, rt * NSEL], F32, tag="s6")
            a = mid.tile([P, rt * NSEL], F32, tag="a")
            if act_order == "sq_first":
                # s6 = 0.6 * d^2   (ACT, scale applied before Square)
                nc.scalar.activation(s6[:], dflat, ACTF.Square, scale=s6scale)
                # a = |d|   (ScalarE Abs)
                nc.scalar.activation(a[:], dflat, ACTF.Abs)
            else:
                # Abs/Ln first: lc reaches the PE sooner; Square's consumer
                # (the f STT) runs one wave later anyway under sw_pipe
                nc.scalar.activation(a[:], dflat, ACTF.Abs)

            if eps_ln:
                # l = Ln(1 + 2^-23 - |d|): the epsilon keeps the argument
                # positive (p=0 -> l ~ -15.9 instead of -inf), making the
                # -100 clamp unnecessary (error impact ~3e-6 of the total).
                # Write bf16 directly for the matmul.
                l = mid.tile([P, rt * NSEL], BF16, tag="l")
                nc.scalar.activation(
                    l[:], a[:], ACTF.Ln, bias=ceps[:, 0:1], scale=-1.0
                )
            else:
                # l = Ln(1 - |d|)
                l = mid.tile([P, rt * NSEL], F32, tag="l")
                nc.scalar.activation(l[:], a[:], ACTF.Ln, bias=1.0, scale=-1.0)

            if act_order != "sq_first":
                nc.scalar.activation(s6[:], dflat, ACTF.Square, scale=s6scale)

            if variant == "acts":
                return None

            return (rt, tt, s6, l)

        def emit_back(state):
            """DVE/PE ops that consume ACT results (run one tile later when
            sw_pipe so the in-order DVE queue never waits on same-tile ACT)."""
            nonlocal chunk_idx
            rt, tt, s6, l = state
            ttflat = tt.rearrange("p n c -> p (n c)")

            # f = (t + 1/3) * s6 = (0.2 + 0.6t) * d^2  (dtype matches lc)
            fdt = F8 if pe_fp8 else BF16
            f = mid.tile([P, rt * NSEL], F16 if relu_clamp else fdt, tag="f")
            if f_engine == "pool2":
                # STT is not ISA-valid on Pool; split into TS + TT there
                tp13 = mid.tile([P, rt * NSEL], F32, tag="tp13")
                nc.gpsimd.tensor_scalar(tp13[:], ttflat, 1.0 / 3.0, None, ALU.add)
                nc.gpsimd.tensor_tensor(f[:], tp13[:], s6[:], ALU.mult)
            else:
                f_eng = nc.gpsimd if f_engine == "gpsimd" else nc.vector
                f_eng.scalar_tensor_tensor(
                    f[:], ttflat, 1.0 / 3.0, s6[:], ALU.add, ALU.mult
                )

            if eps_ln:
                lc = l  # already clamped-by-construction, bf16
            elif relu_clamp:
                # lc' = relu(l + 100) = max(l,-100) + 100 on ScalarE (fp16);
                # the +100 is removed exactly via G2 = column sums of f
                lc = mid.tile([P, rt * NSEL], F16, tag="lc")
                nc.scalar.activation(lc[:], l[:], ACTF.Relu, bias=c100[:, 0:1], scale=1.0)
            else:
                # lc = max(l, -100)  -> bf16 for the matmul
                lc = mid.tile([P, rt * NSEL], BF16, tag="lc")
                if clamp_engine == "gpsimd":
                    nc.gpsimd.tensor_scalar_max(lc[:], l[:], -100.0)
                else:
                    nc.vector.tensor_scalar(lc[:], l[:], -100.0, None, ALU.max)

            if variant == "no_pe":
                return

            # G += f_chunk^T @ lc_chunk over chunks of U row-groups
            us = []
            rem = rt
            while rem > 0:
                u_ = min(u_groups, rem)
                us.append(u_)
                rem -= u_
            if us[-1] < u_groups and chunk_idx + len(us) == total_chunks:
                # stop flag must land on a full-width chunk so the whole
                # PSUM accumulation group closes before the G copy
                us = us[::-1]
            done = 0
            for u in us:
                m = NSEL * u
                sl = slice(done * NSEL, (done + u) * NSEL)
                if split_chunk is not None and chunk_idx >= split_chunk:
                    tgt = Gb
                    st_flag = chunk_idx == split_chunk
                    sp_flag = chunk_idx == total_chunks - 1
                else:
                    tgt = G
                    st_flag = chunk_idx == 0
                    sp_flag = chunk_idx == (
                        total_chunks - 1 if split_chunk is None else split_chunk - 1
                    )
                nc.tensor.matmul(
                    out=tgt[0:m, 0:m],
                    lhsT=f[:, sl],
                    rhs=lc[:, sl],
                    start=st_flag,
                    stop=sp_flag,
                )
                if relu_clamp:
                    nc.tensor.matmul(
                        out=G2[0:m, 0:1],
                        lhsT=f[:, sl],
                        rhs=ones1[:, 0:1],
                        start=(chunk_idx == 0),
                        stop=(chunk_idx == total_chunks - 1),
                    )
                chunk_idx += 1
                done += u
            if (
                split_chunk is not None
                and chunk_idx - len(us) < split_chunk <= chunk_idx
            ):
                # G is closed: ship its half now, overlapped with tail tiles
                g_copy(Gs[:, 0:gdim], G[:])
                nc.sync.dma_start(
                    out=gout.ap()[:, 0:gdim], in_=Gs[:, 0:gdim]
                )

        row0 = 0
        lag = int(sw_pipe)
        pend = []
        for ti, rt in enumerate(tile_sizes):
            rsl = slice(row0, row0 + rt)
            row0 += rt
            st = emit_front(rt, rsl, ti)
            if st is None:
                continue
            if lag == 0:
                emit_back(st)
            else:
                pend.append(st)
                if len(pend) > lag:
                    emit_back(pend.pop(0))
        for st in pend:
            emit_back(st)

        # epilogue: ship G (and the f column sums) to the host
        if variant in ("full", "decoupled"):
            if split_g:
                g_copy(Gs[:, gdim : 2 * gdim], Gb[:])
                nc.sync.dma_start(
                    out=gout.ap()[:, gdim : 2 * gdim],
                    in_=Gs[:, gdim : 2 * gdim],
                )
            else:
                g_copy(Gs[:, 0:gdim], G[:])
                if relu_clamp:
                    g_copy(Gs[:, gdim : gdim + 1], G2[:])
                nc.sync.dma_start(out=gout.ap(), in_=Gs[:])
        else:
            nc.vector.memset(Gs[:], 0.0)
            nc.sync.dma_start(out=gout.ap(), in_=Gs[:])

    nc.compile()
    return nc


_NC = None


TAPER = [128] * 7 + [64, 48, 16]  # smaller final tiles shorten the compute tail


BEST_KW = dict(
    tile_sizes=TAPER,
    label_cast="none",
    sw_pipe=True,
    eps_ln=True,
    lean="dve",
    io_bufs=3,
    split_g=3,
)


def _get_nc():
    global _NC
    if _NC is None:
        # HWDGE label loads (mixed-dtype i32 reads on DVE) + one-tile software
        # pipeline so the in-order DVE queue never waits on same-tile ACT
        # output + epsilon-biased Ln that makes the -100 clamp unnecessary
        # + lean: the ACT Abs pass replaced by one DVE STT (|d|/2 = (t-.5)*d,
        # the *2 absorbed into Ln's prestage scale) + triple-buffered io DMA
        _NC = build_nc(**BEST_KW)
    return _NC


def kernel(pred_all: np.ndarray, label: np.ndarray) -> np.ndarray:
    assert pred_all.shape == (B, NCOL) and label.shape == (B, NSEL)
    nc = _get_nc()
    pred_all = np.ascontiguousarray(pred_all, dtype=np.float32)
    label = np.ascontiguousarray(label, dtype=np.int32)
    in_maps = [
        {
            "pred": pred_all[c * B_SHARD : (c + 1) * B_SHARD],
            "label": label[c * B_SHARD : (c + 1) * B_SHARD],
        }
        for c in range(N_CORES)
    ]
    r = run_bass_kernel_spmd(nc, in_maps, list(range(N_CORES)))
    total = 0.0
    for c in range(N_CORES):
        total += g_to_partial(r.results[c]["g_out"])
    return np.float32(total)


def g_to_partial(g: np.ndarray) -> float:
    """Sum of diagonal 18x18 blocks of G, scaled by -1/18. When G carries an
    extra column of f column-sums (relu_clamp), remove the +100 shift."""
    n = g.shape[0]
    halves = [g[:, 0:n]]
    if g.shape[1] >= 2 * n:
        halves.append(g[:, n : 2 * n])
    s = 0.0
    for h in halves:
        for b_ in range(n // NSEL):
            s += float(
                h[b_ * NSEL : (b_ + 1) * NSEL, b_ * NSEL : (b_ + 1) * NSEL].sum()
            )
    if g.shape[1] == GDIM + 1:
        s -= 100.0 * NSEL * float(g[:, GDIM].sum())
    return -s / NSEL


if __name__ == "__main__":
    rng = np.random.default_rng(0)
    p = rng.random((B, NCOL), dtype=np.float32)
    t = rng.integers(0, 2, size=(B, NSEL)).astype(np.int32)
    print(kernel(p, t))



# revision 54
# speedup vs baseline: 1.0673x; 1.0673x over previous
"""AttrPredLoss_40 focal-BCE loss kernel for Trainium2 (8 NeuronCores, data parallel).

Math (per sample row, 18 selected attrs j):
    p   = pred[:, SEL]                      # in (0,1)
    t   = label in {0,1}
    d   = t - p
    # BCE log arg: t==1 -> p, t==0 -> 1-p  ==  1-|d|
    l   = max(ln(1-|d|), -100)              # sum_j l = -18*bce_mean
    # focal weight: (t?0.8:0.2) * (t?1-p:p)^2 == (t+1/3)*0.6*d^2
    f   = (t + 1/3) * 0.6 * d^2
    loss = sum_rows (sum_j f) * (-1/18) * (sum_j l)

Champion pipeline (BEST_KW): one HWDGE ring streams pred+label at ~345 GB/s
(triple-buffered 128-row tiles, tapered tail); per tile
    DVE: d = t - p (6 merged-run subs), a2 = (t-0.5)*d = |d|/2 (one STT —
         the label IS d's sign), f = (t+1/3)*s6 (STT)
    ACT: s6 = Square(sqrt(.6)*d), l = Ln(-2*a2 + 1 + 2^-23)   [bf16 out]
    PE:  G += f_chunk^T @ l_chunk, diagonal 18x18 blocks in PSUM,
         4 row-groups (72 cols) per matmul, one start/stop chain.
The eps-biased Ln replaces the -100 clamp (p=0 -> ln ~ -15.9, error ~3e-6
of the total); ln(1+eps-|d|) arrives bit-identically via a2.

total = sum over diagonal 18x18 blocks of G, * (-1/18), summed over cores.
"""

import math
from contextlib import ExitStack

import numpy as np

import concourse.bacc as bacc
import concourse.bass as bass
import concourse.mybir as mybir
import concourse.tile as tile
from concourse.bass_utils import run_bass_kernel_spmd

F32 = mybir.dt.float32
I32 = mybir.dt.int32
BF16 = mybir.dt.bfloat16
F16 = mybir.dt.float16
F8 = mybir.dt.float8e4
ALU = mybir.AluOpType
ACTF = mybir.ActivationFunctionType

# selected attribute indices, as contiguous runs: (label_col, pred_col, len)
RUNS = [
    (0, 4, 1),
    (1, 8, 2),
    (3, 11, 1),
    (4, 13, 6),
    (10, 20, 1),
    (11, 22, 2),
    (13, 26, 1),
    (14, 29, 3),
    (17, 36, 1),
]
# same 18 columns as 6 ops: (label_col0, pred_col0, nrun, lstep, pstep, len)
# pairs of runs whose spacing is arithmetic on BOTH the label and pred side
MERGED_RUNS = [
    (0, 4, 1, 0, 0, 1),     # {4}
    (1, 8, 2, 10, 14, 2),   # {8,9} + {22,23}
    (3, 11, 2, 10, 15, 1),  # {11} + {26}
    (4, 13, 1, 0, 0, 6),    # {13..18}
    (10, 20, 2, 7, 16, 1),  # {20} + {36}
    (14, 29, 1, 0, 0, 3),   # {29,30,31}
]
NSEL = 18
NCOL = 40

N_CORES = 8
B = 1_048_576
B_SHARD = B // N_CORES  # 131072
P = 128  # SBUF partitions
U = 4  # row-groups per matmul chunk (must divide r; 18*4 = 72 partitions)
GDIM = NSEL * U  # 72


def build_nc(
    b_shard: int = B_SHARD,
    r: int = 128,
    label_cast: str = "dma",
    loop_n: int = 1,
    io_bufs: int = 2,
    variant: str = "full",
    tile_sizes: list | None = None,
    label_bufs: int | None = None,
    clamp_engine: str = "dve",
    label_engine: str = "sync",
    loop_staggered: bool = False,
    sw_pipe: bool = False,
    relu_clamp: bool = False,
    eps_ln: bool = False,
    act_order: str = "sq_first",
    mid_bufs: int = 2,
    lean: bool = False,
    f_engine: str = "dve",
    gcopy_engine: str = "scalar",
    label_prefetch: bool = False,
    s6_bf16: bool = False,
    warm_ln: bool = False,
    tail_label_prefetch: int = 0,
    u_groups: int = U,
    d_psum: bool = False,
    pe_fp8: bool = False,
    split_g: int = 0,
    gout_engine: str = "sync",
):
    """Build the per-core Bass module. b_shard rows per core; r rows per
    partition per tile (or explicit tile_sizes list summing to nrows).
    loop_n>1 wraps the body in a device-side For loop (same data re-read
    each iteration) for wall-clock benchmarking."""
    assert b_shard % P == 0
    nrows = b_shard // P
    if tile_sizes is None:
        assert nrows % r == 0
        tile_sizes = [r] * (nrows // r)
    assert sum(tile_sizes) == nrows, (tile_sizes, nrows)
    assert all(t % u_groups == 0 or t >= u_groups for t in tile_sizes)

    nc = bacc.Bacc("TRN2", target_bir_lowering=False, debug=False)

    pred = nc.dram_tensor("pred", [b_shard, NCOL], F32, kind="ExternalInput")
    label = nc.dram_tensor("label", [b_shard, NSEL], I32, kind="ExternalInput")
    gdim = NSEL * u_groups
    if split_g:
        assert not relu_clamp
        gw = 2 * gdim
    else:
        gw = gdim + 1 if relu_clamp else gdim
    gout = nc.dram_tensor("g_out", [gdim, gw], F32, kind="ExternalOutput")

    # blocked layout: partition p holds rows [p*nrows, (p+1)*nrows)
    pred_r = pred.ap().rearrange("(p n) c -> p n c", p=P)
    label_r = label.ap().rearrange("(p n) c -> p n c", p=P)

    s6scale = math.sqrt(0.6)

    with tile.TileContext(nc) as tc, ExitStack() as ctx:
        io = ctx.enter_context(tc.tile_pool(name="io", bufs=io_bufs))
        if label_bufs is None:
            iol = io
        else:
            iol = ctx.enter_context(tc.tile_pool(name="iol", bufs=label_bufs))
        mid = ctx.enter_context(tc.tile_pool(name="mid", bufs=mid_bufs))
        singles = ctx.enter_context(tc.tile_pool(name="singles", bufs=1))
        psum = ctx.enter_context(tc.tile_pool(name="psum", bufs=1, space="PSUM"))
        psum_d = (
            ctx.enter_context(tc.tile_pool(name="psumd", bufs=1, space="PSUM"))
            if d_psum
            else None
        )

        G = psum.tile([gdim, gdim], F32)
        Gb = None
        if split_g:
            Gb = psum.tile([gdim, gdim], F32, tag="Gb")
        G2 = None
        ones1 = None
        if relu_clamp:
            G2 = psum.tile([gdim, 1], F32)
            ones1 = singles.tile([P, 1], F16)
            nc.vector.memset(ones1[:], 1.0)
            c100 = singles.tile([P, 1], F32)
            nc.vector.memset(c100[:], 100.0)
        ceps = None
        if eps_ln:
            ceps = singles.tile([P, 1], F32)
            nc.vector.memset(ceps[:], 1.0 + 2.0**-23)

        total_chunks = sum(-(-t // u_groups) for t in tile_sizes)
        chunk_idx = 0
        split_chunk = (
            total_chunks - sum(-(-t // u_groups) for t in tile_sizes[-split_g:])
            if split_g
            else None
        )

        if loop_n > 1:
            loop_cm = tc.For_i(0, loop_n, 1, staggered_reset=loop_staggered)
        else:
            loop_cm = None
        if loop_cm is not None:
            ctx.enter_context(loop_cm)

        pt_s = tt_s = None
        if variant == "decoupled":
            # static source tiles: compute runs the full chain but never
            # waits on (or releases) the streaming DMA buffers
            rt_max = max(tile_sizes)
            pt_s = singles.tile([P, rt_max, NCOL], F32, tag="pt_s")
            nc.vector.memset(pt_s[:], 0.5)
            tt_s = singles.tile([P, rt_max, NSEL], I32, tag="tt_s")
            nc.vector.memset(tt_s[:], 1)

        if warm_ln:
            # tiny Ln first: its table set (natural_log) also contains
            # square/abs, so the per-iteration LoadActFuncSet pair for
            # Square-set/Ln-set alternation may collapse to none
            wt = singles.tile([P, 1], F32, tag="warm")
            nc.scalar.activation(wt[:], ceps[:, 0:1], ACTF.Ln)

        tlab = None
        if label_prefetch:
            # all labels in ONE leading DMA: 10 fewer stream boundaries and
            # no per-tile label dependency in the pipeline tail
            tlab = singles.tile([P, nrows, NSEL], I32, tag="tlab")
            lab_eng0 = {"sync": nc.sync, "scalar": nc.scalar, "gpsimd": nc.gpsimd}[
                label_engine
            ]
            lab_eng0.dma_start(out=tlab[:], in_=label_r[:, :, :])

        # prefetch the last K tiles' labels at stream start so the tail
        # chains begin the moment their (last-in-stream) pred DMAs land
        pre_tt: dict[int, object] = {}
        if tail_label_prefetch:
            row0_pre = 0
            offs = []
            for i, rt_ in enumerate(tile_sizes):
                offs.append((i, rt_, row0_pre))
                row0_pre += rt_
            for i, rt_, r0 in offs[-tail_label_prefetch:]:
                ptile = singles.tile([P, rt_, NSEL], I32, tag=f"ttpre{i}")
                nc.sync.dma_start(
                    out=ptile[:], in_=label_r[:, slice(r0, r0 + rt_), :]
                )
                pre_tt[i] = ptile

        Gs = singles.tile([gdim, gw], F32, tag="Gs")

        def g_copy(dst, src):
            if gcopy_engine == "vector":
                nc.vector.tensor_copy(out=dst, in_=src)
            else:
                nc.scalar.copy(dst, src)

        def emit_front(rt, rsl, idx=-1):
            """DMA + ops that depend only on this tile's DMA."""
            pt = io.tile([P, rt, NCOL], F32, tag="pred")
            nc.sync.dma_start(out=pt[:], in_=pred_r[:, rsl, :])

            lab_eng = {"sync": nc.sync, "scalar": nc.scalar, "gpsimd": nc.gpsimd}[
                label_engine
            ]
            if idx in pre_tt:
                tt = pre_tt[idx]
            elif label_prefetch:
                tt = tlab[:, rsl, :]
            elif label_cast == "dma":
                tt = iol.tile([P, rt, NSEL], F32, tag="label")
                nc.gpsimd.dma_start(out=tt[:], in_=label_r[:, rsl, :])
            elif label_cast == "none":
                # no cast: DVE ops read the i32 labels with on-read convert.
                # label_engine picks the issuing queue — a non-sync engine
                # overlaps the label stream with the pred stream on SP.
                tt = iol.tile([P, rt, NSEL], I32, tag="label")
                lab_eng.dma_start(out=tt[:], in_=label_r[:, rsl, :])
            else:
                ti = iol.tile([P, rt, NSEL], I32, tag="label_i")
                nc.sync.dma_start(out=ti[:], in_=label_r[:, rsl, :])
                tt = iol.tile([P, rt, NSEL], F32, tag="label")
                nc.gpsimd.tensor_copy(
                    out=tt.rearrange("p n c -> p (n c)"),
                    in_=ti.rearrange("p n c -> p (n c)"),
                )

            if variant == "dma_only":
                return None

            if variant == "decoupled":
                pt = pt_s[:, 0:rt, :]
                tt = tt_s[:, 0:rt, :]

            # d = t - p on the 18 selected columns (6 merged-run ops)
            d = (psum_d if d_psum else mid).tile([P, rt, NSEL], F32, tag="d")

            def run_view(tile3, col0, nrun, step, ln_):
                base = tile3 if isinstance(tile3, bass.AP) else tile3[:]
                ap = [base.ap[0], base.ap[1]]
                if nrun > 1:
                    ap.append([step, nrun])
                ap.append([1, ln_])
                return bass.AP(tensor=base.tensor, offset=base.offset + col0, ap=ap)

            for lc0, pc0, nrun, lstep, pstep, ln_ in MERGED_RUNS:
                nc.vector.tensor_sub(
                    run_view(d, lc0, nrun, lstep, ln_),
                    run_view(tt, lc0, nrun, lstep, ln_),
                    run_view(pt, pc0, nrun, pstep, ln_),
                )

            if variant == "dpass":
                return None

            dflat = d.rearrange("p n c -> p (n c)")

            if lean:
                # a2 = (t - 0.5)*d = |d|/2 in ONE STT op (the label's value
                # IS d's sign: t=1 -> d>=0, t=0 -> d<=0). Ln's affine
                # prestage absorbs the *2: Ln(-2*a2 + 1 + eps) = Ln(1+eps-|d|)
                # — the ACT Abs pass disappears, numerics unchanged.
                assert eps_ln
                ttf = tt.rearrange("p n c -> p (n c)")
                s6 = mid.tile([P, rt * NSEL], BF16 if s6_bf16 else F32, tag="s6")
                if lean == "dvesq":
                    # s6' = d*d on DVE (ACT down to the single Ln pass);
                    # the 0.6 factor moves to the host-side block sum
                    nc.vector.tensor_tensor(s6[:], dflat, dflat, ALU.mult)
                else:
                    nc.scalar.activation(s6[:], dflat, ACTF.Square, scale=s6scale)
                a2 = mid.tile([P, rt * NSEL], F32, tag="a2")
                a2_eng = nc.gpsimd if lean == "pool" else nc.vector
                a2_eng.scalar_tensor_tensor(
                    a2[:], ttf, -0.5, dflat, ALU.add, ALU.mult
                )
                l = mid.tile([P, rt * NSEL], F8 if pe_fp8 else BF16, tag="l")
                nc.scalar.activation(
                    l[:], a2[:], ACTF.Ln, bias=ceps[:, 0:1], scale=-2.0
                )
                if variant == "acts":
                    return None
                return (rt, tt, s6, l)

            s6 = mid.tile([P, rt * NSEL], F32, tag="s6")
            a = mid.tile([P, rt * NSEL], F32, tag="a")
            if act_order == "sq_first":
                # s6 = 0.6 * d^2   (ACT, scale applied before Square)
                nc.scalar.activation(s6[:], dflat, ACTF.Square, scale=s6scale)
                # a = |d|   (ScalarE Abs)
                nc.scalar.activation(a[:], dflat, ACTF.Abs)
            else:
                # Abs/Ln first: lc reaches the PE sooner; Square's consumer
                # (the f STT) runs one wave later anyway under sw_pipe
                nc.scalar.activation(a[:], dflat, ACTF.Abs)

            if eps_ln:
                # l = Ln(1 + 2^-23 - |d|): the epsilon keeps the argument
                # positive (p=0 -> l ~ -15.9 instead of -inf), making the
                # -100 clamp unnecessary (error impact ~3e-6 of the total).
                # Write bf16 directly for the matmul.
                l = mid.tile([P, rt * NSEL], BF16, tag="l")
                nc.scalar.activation(
                    l[:], a[:], ACTF.Ln, bias=ceps[:, 0:1], scale=-1.0
                )
            else:
                # l = Ln(1 - |d|)
                l = mid.tile([P, rt * NSEL], F32, tag="l")
                nc.scalar.activation(l[:], a[:], ACTF.Ln, bias=1.0, scale=-1.0)

            if act_order != "sq_first":
                nc.scalar.activation(s6[:], dflat, ACTF.Square, scale=s6scale)

            if variant == "acts":
                return None

            return (rt, tt, s6, l)

        def emit_back(state):
            """DVE/PE ops that consume ACT results (run one tile later when
            sw_pipe so the in-order DVE queue never waits on same-tile ACT)."""
            nonlocal chunk_idx
            rt, tt, s6, l = state
            ttflat = tt.rearrange("p n c -> p (n c)")

            # f = (t + 1/3) * s6 = (0.2 + 0.6t) * d^2  (dtype matches lc)
            fdt = F8 if pe_fp8 else BF16
            f = mid.tile([P, rt * NSEL], F16 if relu_clamp else fdt, tag="f")
            if f_engine == "pool2":
                # STT is not ISA-valid on Pool; split into TS + TT there
                tp13 = mid.tile([P, rt * NSEL], F32, tag="tp13")
                nc.gpsimd.tensor_scalar(tp13[:], ttflat, 1.0 / 3.0, None, ALU.add)
                nc.gpsimd.tensor_tensor(f[:], tp13[:], s6[:], ALU.mult)
            else:
                f_eng = nc.gpsimd if f_engine == "gpsimd" else nc.vector
                f_eng.scalar_tensor_tensor(
                    f[:], ttflat, 1.0 / 3.0, s6[:], ALU.add, ALU.mult
                )

            if eps_ln:
                lc = l  # already clamped-by-construction, bf16
            elif relu_clamp:
                # lc' = relu(l + 100) = max(l,-100) + 100 on ScalarE (fp16);
                # the +100 is removed exactly via G2 = column sums of f
                lc = mid.tile([P, rt * NSEL], F16, tag="lc")
                nc.scalar.activation(lc[:], l[:], ACTF.Relu, bias=c100[:, 0:1], scale=1.0)
            else:
                # lc = max(l, -100)  -> bf16 for the matmul
                lc = mid.tile([P, rt * NSEL], BF16, tag="lc")
                if clamp_engine == "gpsimd":
                    nc.gpsimd.tensor_scalar_max(lc[:], l[:], -100.0)
                else:
                    nc.vector.tensor_scalar(lc[:], l[:], -100.0, None, ALU.max)

            if variant == "no_pe":
                return

            # G += f_chunk^T @ lc_chunk over chunks of U row-groups
            us = []
            rem = rt
            while rem > 0:
                u_ = min(u_groups, rem)
                us.append(u_)
                rem -= u_
            if us[-1] < u_groups and chunk_idx + len(us) == total_chunks:
                # stop flag must land on a full-width chunk so the whole
                # PSUM accumulation group closes before the G copy
                us = us[::-1]
            done = 0
            for u in us:
                m = NSEL * u
                sl = slice(done * NSEL, (done + u) * NSEL)
                if split_chunk is not None and chunk_idx >= split_chunk:
                    tgt = Gb
                    st_flag = chunk_idx == split_chunk
                    sp_flag = chunk_idx == total_chunks - 1
                else:
                    tgt = G
                    st_flag = chunk_idx == 0
                    sp_flag = chunk_idx == (
                        total_chunks - 1 if split_chunk is None else split_chunk - 1
                    )
                nc.tensor.matmul(
                    out=tgt[0:m, 0:m],
                    lhsT=f[:, sl],
                    rhs=lc[:, sl],
                    start=st_flag,
                    stop=sp_flag,
                )
                if relu_clamp:
                    nc.tensor.matmul(
                        out=G2[0:m, 0:1],
                        lhsT=f[:, sl],
                        rhs=ones1[:, 0:1],
                        start=(chunk_idx == 0),
                        stop=(chunk_idx == total_chunks - 1),
                    )
                chunk_idx += 1
                done += u
            if (
                split_chunk is not None
                and chunk_idx - len(us) < split_chunk <= chunk_idx
            ):
                # G is closed: ship its half now, overlapped with tail tiles
                g_copy(Gs[:, 0:gdim], G[:])
                g_dma_eng = nc.scalar if gout_engine == "scalar" else nc.sync
                g_dma_eng.dma_start(
                    out=gout.ap()[:, 0:gdim], in_=Gs[:, 0:gdim]
                )

        row0 = 0
        lag = int(sw_pipe)
        pend = []
        for ti, rt in enumerate(tile_sizes):
            rsl = slice(row0, row0 + rt)
            row0 += rt
            st = emit_front(rt, rsl, ti)
            if st is None:
                continue
            if lag == 0:
                emit_back(st)
            else:
                pend.append(st)
                if len(pend) > lag:
                    emit_back(pend.pop(0))
        for st in pend:
            emit_back(st)

        # epilogue: ship G (and the f column sums) to the host
        if variant in ("full", "decoupled"):
            if split_g:
                g_copy(Gs[:, gdim : 2 * gdim], Gb[:])
                g_dma_eng2 = nc.scalar if gout_engine == "scalar" else nc.sync
                g_dma_eng2.dma_start(
                    out=gout.ap()[:, gdim : 2 * gdim],
                    in_=Gs[:, gdim : 2 * gdim],
                )
            else:
                g_copy(Gs[:, 0:gdim], G[:])
                if relu_clamp:
                    g_copy(Gs[:, gdim : gdim + 1], G2[:])
                nc.sync.dma_start(out=gout.ap(), in_=Gs[:])
        else:
            nc.vector.memset(Gs[:], 0.0)
            nc.sync.dma_start(out=gout.ap(), in_=Gs[:])

    nc.compile()
    return nc


_NC = None


TAPER = [128] * 7 + [64, 48, 16]  # smaller final tiles shorten the compute tail


BEST_KW = dict(
    tile_sizes=TAPER,
    label_cast="none",
    sw_pipe=True,
    eps_ln=True,
    lean="dve",
    io_bufs=3,
    split_g=3,
    gout_engine="scalar",
)


def _get_nc():
    global _NC
    if _NC is None:
        # HWDGE label loads (mixed-dtype i32 reads on DVE) + one-tile software
        # pipeline so the in-order DVE queue never waits on same-tile ACT
        # output + epsilon-biased Ln that makes the -100 clamp unnecessary
        # + lean: the ACT Abs pass replaced by one DVE STT (|d|/2 = (t-.5)*d,
        # the *2 absorbed into Ln's prestage scale) + triple-buffered io DMA
        _NC = build_nc(**BEST_KW)
    return _NC


def kernel(pred_all: np.ndarray, label: np.ndarray) -> np.ndarray:
    assert pred_all.shape == (B, NCOL) and label.shape == (B, NSEL)
    nc = _get_nc()
    pred_all = np.ascontiguousarray(pred_all, dtype=np.float32)
    label = np.ascontiguousarray(label, dtype=np.int32)
    in_maps = [
        {
            "pred": pred_all[c * B_SHARD : (c + 1) * B_SHARD],
            "label": label[c * B_SHARD : (c + 1) * B_SHARD],
        }
        for c in range(N_CORES)
    ]
    r = run_bass_kernel_spmd(nc, in_maps, list(range(N_CORES)))
    total = 0.0
    for c in range(N_CORES):
        total += g_to_partial(r.results[c]["g_out"])
    return np.float32(total)


def g_to_partial(g: np.ndarray) -> float:
    """Sum of diagonal 18x18 blocks of G, scaled by -1/18. When G carries an
    extra column of f column-sums (relu_clamp), remove the +100 shift."""
    n = g.shape[0]
    halves = [g[:, 0:n]]
    if g.shape[1] >= 2 * n:
        halves.append(g[:, n : 2 * n])
    s = 0.0
    for h in halves:
        for b_ in range(n // NSEL):
            s += float(
                h[b_ * NSEL : (b_ + 1) * NSEL, b_ * NSEL : (b_ + 1) * NSEL].sum()
            )
    if g.shape[1] == GDIM + 1:
        s -= 100.0 * NSEL * float(g[:, GDIM].sum())
    return -s / NSEL


if __name__ == "__main__":
    rng = np.random.default_rng(0)
    p = rng.random((B, NCOL), dtype=np.float32)
    t = rng.integers(0, 2, size=(B, NSEL)).astype(np.int32)
    print(kernel(p, t))

